# revision 12
# baseline (speedup 1.0000x reference)
"""GRU (hidden_size=1) kernel for Trainium2, data-parallel over batch on 8 cores.

v4: x-stationary production + Newton-linearized tensor_tensor_scan.

Per core (B_loc = 256 = 128 partitions x 2 column-halves h):
  - host stages x as fp8 xt[chunk, dpart, t, k, h, b] (8KB contiguous per
    partition per chunk); w_ih split-precision: w6[dpart, k, 0:3] = fp8(w),
    w6[dpart, k, 3:6] = fp8((w - fp8(w)) * 16).
  - production: per (t, h): 4 matmuls with x[128d, 128b] fp8 as the
    STATIONARY operand and w6[:, k, :] moving (N=6), accumulating into
    psum[128b, 6*G] (G=16 t per group). h0/h1 go to different PSUM banks
    and the (k, h) loop order alternates banks every matmul so drains
    overlap. Batch lands directly in psum partitions: no PE transposes.
  - drain per (group, h): 2 DVE ops: scratch = (res * 1/16) + bias48;
    gi[g, h, t] = scratch + hi  (cast to bf16 planes gi[128, 3, 2, T]).
  - scan: Newton-linearized fixed point. Per window [t0, t1) one
    iteration: gates at lagged h; linear coefficients
      a = z + w1 * z(1-z) * (h - n),   c = n(1-z) - w1 * z(1-z)(h-n) * h
    then ONE tensor_tensor_scan per h-half solves h_t = a_t h_{t-1} + c_t
    exactly (fp32 internal state). Windows follow production (span 3
    groups); 3 suffix windows converge the tail. 3-4 iterations per
    position suffice (quadratic-ish convergence; w1 z-term dominates F').
  - batch-sum of H via ones-matmul; host sums cores / divides by B.
"""

import numpy as np

import concourse.bass as bass
import concourse.mybir as mybir
from concourse.bass_types import AP
from concourse.tile import TileContext
from concourse.bass_utils import run_bass_kernel_spmd

F32 = mybir.dt.float32
BF16 = mybir.dt.bfloat16
FP8 = mybir.dt.float8e4
AF = mybir.ActivationFunctionType
ALU = mybir.AluOpType

N_CORES = 8
B, T, D = 2048, 128, 512
B_LOC = B // N_CORES          # 256
NH = 2                        # column halves of the local batch
NCH = D // 128                # 4 contraction chunks
TPC = 16                      # timesteps per DMA chunk
NCHUNK = T // TPC             # 8
G = 16                        # timesteps per psum flush group (1 chunk)
NGRP = T // G                 # 8
HSTR = T + 2                  # H tile half-stride (even, for 4B alignment)
VMAX = 64                     # max scan window width
SPAN = 4                      # production window = last SPAN flushed groups
TAILS = (96, 112)             # suffix windows after the last flush

_CACHE = {}


def build_nc(w0, w1, w2, bh2):
    nc = bass.Bass(trn_type="TRN2")

    xt = nc.dram_tensor("xt", [NCHUNK, 128, TPC, NCH, NH, 128], FP8,
                        kind="ExternalInput")
    w6 = nc.dram_tensor("w6", [128, NCH, 6], FP8, kind="ExternalInput")
    cst = nc.dram_tensor("cst", [128, 52], F32, kind="ExternalInput")
    out = nc.dram_tensor("out", [1, 2 * T], F32, kind="ExternalOutput")

    with TileContext(nc) as tc:
        with (
            tc.tile_pool(name="xpool", bufs=4) as xpool,
            tc.tile_pool(name="consts", bufs=1) as consts,
            tc.tile_pool(name="scan", bufs=1) as scan,
            tc.tile_pool(name="drn", bufs=2) as drn,
            tc.tile_pool(name="swp", bufs=3) as swp,
            tc.tile_pool(name="pmm", bufs=4, space="PSUM") as pmm,
            tc.tile_pool(name="psum2", bufs=1, space="PSUM") as ps2,
        ):
            # ---- constants ----
            # const DMAs ride the ACT HWDGE queue so the Sync queue's
            # FIFO starts with x chunk 0 (transfers per queue are serial)
            w6_sb = consts.tile([128, NCH, 6], FP8)
            nc.scalar.dma_start(out=w6_sb, in_=w6[:])
            cst_sb = consts.tile([128, 52], F32)
            nc.scalar.dma_start(out=cst_sb, in_=cst[:])
            bias48 = cst_sb[:, 0:48]           # cols 3j+g = bias[g]
            h0_sb = cst_sb[:, 48:50]
            ones_sb = consts.tile([128, 1], BF16)
            nc.vector.memset(ones_sb, 1.0)

            # ---- x DMA helper: [128, TPC, NCH, NH, 128] fp8, 8KB/partition
            def dma_chunk(c):
                x_sb = xpool.tile([128, TPC, NCH, NH, 128], FP8, name="x_sb")
                nc.sync.dma_start(out=x_sb, in_=xt[c])
                return x_sb

            # prefetch exactly 1 chunk so chunk 0 gets the full DMA
            # bandwidth (concurrent transfers round-robin at packet
            # granularity and would delay the first matmul)
            x_tiles = {0: dma_chunk(0)}

            # warm-up consumers of const DMAs (absorb semaphores)
            warm_sb = consts.tile([3, 1], F32)
            nc.scalar.copy(warm_sb, cst_sb[0:3, 50:51])
            warm_dv = consts.tile([3, 1], F32)
            nc.vector.tensor_copy(warm_dv, cst_sb[0:3, 50:51])
            warm_pl = consts.tile([3, 1], F32)
            nc.gpsimd.tensor_copy(warm_pl, cst_sb[0:3, 50:51])

            # ---- persistent buffers ----
            # gi planes [128, 3, NH, T] bf16; H [128, NH*(T+2)] bf16,
            # H col h*HSTR + j holds h_{j-1} (col 0 = h0).
            gi = scan.tile([128, 3, NH, T], BF16)
            H = scan.tile([128, NH * HSTR], BF16)
            nc.gpsimd.memset(H, 0.0)
            h0v = AP(tensor=H.tensor, offset=H.offset, ap=[H.ap[0], [HSTR, NH]])
            nc.vector.tensor_copy(h0v, h0_sb)

            def hview(t0, V):
                # lagged-h view [128, NH, V]: h_{t-1} for t in [t0, t0+V)
                return AP(tensor=H.tensor, offset=H.offset + t0,
                          ap=[H.ap[0], [HSTR, NH], [1, V]])

            def gview(g, t0, V):
                # gate plane view [128, NH, V]
                return AP(tensor=gi.tensor, offset=gi.offset + g * NH * T + t0,
                          ap=[gi.ap[0], [T, NH], [1, V]])

            # ---- production ----
            ps_of = {}     # group -> [ps_h0, ps_h1] (separate banks)

            def produce_chunk(c):
                x_sb = x_tiles.pop(c)
                grp = c
                ps_of[grp] = [
                    pmm.tile([128, 512], F32, tag="pmm", name=f"ps{grp}h{h}")
                    for h in range(NH)
                ]
                pss = ps_of[grp]
                for tt in range(TPC):
                    j = tt
                    for k in range(NCH):
                        for h in range(NH):
                            nc.tensor.matmul(
                                pss[h][:, 6 * j:6 * j + 6],
                                x_sb[:, tt, k, h, :],
                                w6_sb[:, k, :],
                                start=(k == 0),
                                stop=(k == NCH - 1),
                            )

            def flush_group(grp):
                pss = ps_of.pop(grp)
                for h in range(NH):
                    ps = pss[h]
                    hi = AP(tensor=ps.tensor, offset=ps.offset,
                            ap=[ps.ap[0], [1, 3], [6, G]])
                    res = AP(tensor=ps.tensor, offset=ps.offset + 3,
                             ap=[ps.ap[0], [1, 3], [6, G]])
                    bias_v = AP(tensor=bias48.tensor, offset=bias48.offset,
                                ap=[bias48.ap[0], [1, 3], [3, G]])
                    sc = drn.tile([128, 48], F32, name="drsc")
                    scv = AP(tensor=sc.tensor, offset=sc.offset,
                             ap=[sc.ap[0], [1, 3], [3, G]])
                    nc.vector.scalar_tensor_tensor(
                        scv, res, 1.0 / 16, bias_v, op0=ALU.mult, op1=ALU.add)
                    giv = AP(tensor=gi.tensor,
                             offset=gi.offset + h * T + G * grp,
                             ap=[gi.ap[0], [NH * T, 3], [1, G]])
                    nc.vector.tensor_tensor(giv, scv, hi, op=ALU.add)

            # ---- one Newton-linearized iteration over window [t0, t1) ----
            def iterate(t0, t1):
                V = t1 - t0
                HL = hview(t0, V)

                def half3(tile):
                    return AP(tensor=tile.tensor, offset=tile.offset,
                              ap=[tile.ap[0], [V, NH], [1, V]])

                srz = swp.tile([128, 4 * VMAX], BF16, tag="srz", name="srz")
                srv = AP(tensor=srz.tensor, offset=srz.offset,
                         ap=[srz.ap[0], [V, NH], [1, V]])
                szv = AP(tensor=srz.tensor, offset=srz.offset + 2 * V,
                         ap=[srz.ap[0], [V, NH], [1, V]])
                nc.vector.scalar_tensor_tensor(
                    srv, HL, w0, gview(0, t0, V), op0=ALU.mult, op1=ALU.add)
                nc.vector.scalar_tensor_tensor(
                    szv, HL, w1, gview(1, t0, V), op0=ALU.mult, op1=ALU.add)
                rz = swp.tile([128, 4 * VMAX], BF16, tag="rz", name="rz")
                nc.scalar.activation(out=rz[:, 0:4 * V], in_=srz[:, 0:4 * V],
                                     func=AF.Sigmoid)
                # flat [128, 2V] views (step-1 bf16 -> DVE 2x mode); the
                # [NH, V] layouts of these tiles are contiguous in cols
                rf = rz[:, 0:2 * V]
                zf = rz[:, 2 * V:4 * V]

                def t2(tag):
                    tl = swp.tile([128, 2 * VMAX], BF16, tag=tag, name=tag)
                    return tl[:, 0:2 * V], half3(tl)

                gh2f, gh2 = t2("gh2")
                nc.gpsimd.tensor_scalar(gh2, HL, w2, bh2,
                                        op0=ALU.mult, op1=ALU.add)
                nrf, nr = t2("nr")
                nc.gpsimd.tensor_tensor(nrf, rf, gh2f, op=ALU.mult)
                npf, npre = t2("npre")
                nc.gpsimd.tensor_tensor(npre, nr, gview(2, t0, V), op=ALU.add)
                ntf, nt = t2("nt")
                nc.scalar.activation(out=ntf, in_=npf, func=AF.Tanh)
                # u = z(1-z)  (z - z^2)
                zzf, zz = t2("zz")
                nc.gpsimd.tensor_tensor(zzf, zf, zf, op=ALU.mult)
                uf, u = t2("u")
                nc.gpsimd.tensor_tensor(uf, zf, zzf, op=ALU.subtract)
                # d = h - n; p = u*d
                df, d = t2("d")
                nc.vector.tensor_tensor(d, HL, nt, op=ALU.subtract)
                pf, p = t2("p")
                nc.vector.tensor_tensor(pf, uf, df, op=ALU.mult)
                phf, pH = t2("pH")
                nc.vector.tensor_tensor(pH, p, HL, op=ALU.mult)
                # a = z + w1*p
                af, av = t2("a")
                nc.vector.scalar_tensor_tensor(af, pf, w1, zf,
                                               op0=ALU.mult, op1=ALU.add)
                # c = n(1-z) - w1*pH
                c1f, c1 = t2("c1")
                nc.gpsimd.tensor_tensor(c1f, zf, ntf, op=ALU.mult)
                c2f, c2 = t2("c2")
                nc.vector.tensor_tensor(c2f, ntf, c1f, op=ALU.subtract)
                cf, cv = t2("c")
                nc.vector.scalar_tensor_tensor(cf, phf, -w1, c2f,
                                               op0=ALU.mult, op1=ALU.add)
                # per-half linear scan: h_t = a_t * h_{t-1} + c_t
                for h in range(NH):
                    nc.vector.tensor_tensor_scan(
                        AP(tensor=H.tensor,
                           offset=H.offset + h * HSTR + t0 + 1,
                           ap=[H.ap[0], [1, V]]),
                        af[:, h * V:(h + 1) * V],
                        cf[:, h * V:(h + 1) * V],
                        AP(tensor=H.tensor, offset=H.offset + h * HSTR + t0,
                           ap=[H.ap[0], [1, 1]]),
                        op0=ALU.mult,
                        op1=ALU.add,
                    )

            # ---- schedule ----
            for c in range(NCHUNK):
                if c + 1 < NCHUNK:
                    x_tiles[c + 1] = dma_chunk(c + 1)
                produce_chunk(c)
                grp = c
                flush_group(grp)
                if grp < NGRP - 1:
                    t1 = G * (grp + 1)
                    iterate(max(0, t1 - G * SPAN), t1)
                if grp == NGRP - 2:
                    # extra catch-up window, hidden under production
                    iterate(64, 112)
            for t0 in TAILS:
                iterate(t0, T)

            # ---- batch-sum over partitions ----
            hv = AP(tensor=H.tensor, offset=H.offset + 1,
                    ap=[H.ap[0], [HSTR, NH], [1, T]])
            sum_ps = ps2.tile([1, 2 * T], F32, name="sum_ps")
            nc.tensor.matmul(sum_ps, ones_sb, hv, start=True, stop=True)
            sum_sb = scan.tile([1, 2 * T], F32)
            nc.vector.tensor_copy(sum_sb, sum_ps)
            nc.sync.dma_start(out=out[:], in_=sum_sb)

    _strip_same_engine_waits(nc)
    return nc


_ENG_PFX = {
    mybir.EngineType.Activation: "Activation",
    mybir.EngineType.DVE: "DVE",
    mybir.EngineType.PE: "PE",
    mybir.EngineType.Pool: "Pool",
    mybir.EngineType.SP: "SP",
}


def _strip_same_engine_waits(nc):
    """The compute-engine instruction formats have a single sync-wait slot.

    Tile's semaphore assignment is not transitively minimal and often adds a
    wait on the instruction's own engine semaphore next to a cross-engine
    wait. Engines execute their own stream in order, so same-engine waits
    are vacuous -- drop them when an instruction carries more than one wait.
    """
    multi = []
    for inst in nc.inst_map.values():
        si = inst.sync_info
        if not si or not si.on_wait or len(si.on_wait) <= 1:
            continue
        pfx = _ENG_PFX.get(inst.engine)
        if pfx is not None:
            kept = [
                w
                for w in si.on_wait
                if not (w.ant_name or "").startswith(pfx + "_")
            ]
            if len(kept) != len(si.on_wait):
                si.on_wait = kept
        if len(si.on_wait) > 1:
            multi.append((inst.name, type(inst).__name__, str(inst.engine),
                          [w.ant_name for w in si.on_wait]))

    # Any instruction still carrying >1 wait cannot encode (single HW wait
    # slot): hoist all but one wait onto single-wait InstDrains inserted
    # just before it on the same engine.
    for block in nc.m.functions[0].blocks:
        insts = block.instructions
        for idx in range(len(insts) - 1, -1, -1):
            inst = insts[idx]
            si = inst.sync_info
            if not si or not si.on_wait or len(si.on_wait) <= 1:
                continue
            waits = list(si.on_wait)
            si.on_wait = waits[-1:]
            pre = []
            for k, w in enumerate(waits[:-1]):
                d = mybir.InstDrain(
                    name=f"{inst.name}-w{k}", ins=[], outs=[]
                )
                d.engine = inst.engine
                d.sync_info = mybir.SyncInfo(on_wait=[w], on_update=[])
                pre.append(d)
            insts[idx:idx] = pre
            multi = [m for m in multi if m[0] != inst.name]

    if multi:
        import sys
        print(f"[kernel] WARNING: {len(multi)} instructions still have >1 "
              f"sync wait: {multi[:8]}", file=sys.stderr)


def kernel(x, h0, w_ih, w_hh, b_ih, b_hh):
    import ml_dtypes
    fp8 = ml_dtypes.float8_e4m3fn

    x = np.asarray(x, dtype=np.float32)
    h0 = np.asarray(h0, dtype=np.float32)
    w_ih = np.asarray(w_ih, dtype=np.float32)
    w_hh = np.asarray(w_hh, dtype=np.float32)
    b_ih = np.asarray(b_ih, dtype=np.float32)
    b_hh = np.asarray(b_hh, dtype=np.float32)

    w0, w1, w2 = (float(v) for v in w_hh[:, 0])
    bh0, bh1, bh2 = (float(v) for v in b_hh)
    key = (w0, w1, w2, bh2)
    if _CACHE.get("key") != key:
        _CACHE["nc"] = build_nc(w0, w1, w2, bh2)
        _CACHE["key"] = key
    nc = _CACHE["nc"]

    # w6[p, k, 0:3] = fp8(w_ih[g, 128k+p]); w6[p, k, 3:6] = fp8(16*residual)
    w_hi = w_ih.astype(fp8).astype(np.float32)
    w_res = ((w_ih - w_hi) * 16.0).astype(fp8).astype(np.float32)
    w6 = np.zeros((128, NCH, 6), dtype=np.float32)
    w6[:, :, 0:3] = w_hi.T.reshape(NCH, 128, 3).transpose(1, 0, 2)
    w6[:, :, 3:6] = w_res.T.reshape(NCH, 128, 3).transpose(1, 0, 2)
    w6 = w6.astype(fp8)

    bias3 = np.array([b_ih[0] + bh0, b_ih[1] + bh1, b_ih[2]], dtype=np.float32)

    in_maps = []
    for c in range(N_CORES):
        xs = x[c * B_LOC:(c + 1) * B_LOC]                 # [B_loc, T, D]
        # [chunk, dpart, t, k, h, b]; xs b_loc index = 128h + b
        xtb = np.ascontiguousarray(
            xs.reshape(NH, 128, NCHUNK, TPC, NCH, 128)
            .transpose(2, 5, 3, 4, 0, 1)
        ).astype(fp8)
        h0c = h0[0, c * B_LOC:(c + 1) * B_LOC, 0]         # [B_loc]
        h0t = h0c.reshape(NH, 128).T                      # [128, NH]
        cstc = np.zeros((128, 52), dtype=np.float32)
        cstc[:, 0:48] = np.tile(bias3, 16)
        cstc[:, 48:50] = h0t
        in_maps.append({"xt": xtb, "w6": w6, "cst": cstc})

    res = run_bass_kernel_spmd(nc, in_maps, core_ids=list(range(N_CORES)))
    total = np.zeros((2 * T,), dtype=np.float64)
    for r in res.results:
        total += r["out"].reshape(-1).astype(np.float64)
    out = total.reshape(NH, T).sum(axis=0) / B
    return out.astype(np.float32)


# revision 13
# speedup vs baseline: 1.0412x; 1.0412x over previous
"""GRU (hidden_size=1) kernel for Trainium2, data-parallel over batch on 8 cores.

v4: x-stationary production + Newton-linearized tensor_tensor_scan.

Per core (B_loc = 256 = 128 partitions x 2 column-halves h):
  - host stages x as fp8 xt[chunk, dpart, t, k, h, b] (8KB contiguous per
    partition per chunk); w_ih split-precision: w6[dpart, k, 0:3] = fp8(w),
    w6[dpart, k, 3:6] = fp8((w - fp8(w)) * 16).
  - production: per (t, h): 4 matmuls with x[128d, 128b] fp8 as the
    STATIONARY operand and w6[:, k, :] moving (N=6), accumulating into
    psum[128b, 6*G] (G=16 t per group). h0/h1 go to different PSUM banks
    and the (k, h) loop order alternates banks every matmul so drains
    overlap. Batch lands directly in psum partitions: no PE transposes.
  - drain per (group, h): 2 DVE ops: scratch = (res * 1/16) + bias48;
    gi[g, h, t] = scratch + hi  (cast to bf16 planes gi[128, 3, 2, T]).
  - scan: Newton-linearized fixed point. Per window [t0, t1) one
    iteration: gates at lagged h; linear coefficients
      a = z + w1 * z(1-z) * (h - n),   c = n(1-z) - w1 * z(1-z)(h-n) * h
    then ONE tensor_tensor_scan per h-half solves h_t = a_t h_{t-1} + c_t
    exactly (fp32 internal state). Windows follow production (span 3
    groups); 3 suffix windows converge the tail. 3-4 iterations per
    position suffice (quadratic-ish convergence; w1 z-term dominates F').
  - batch-sum of H via ones-matmul; host sums cores / divides by B.
"""

import numpy as np

import concourse.bass as bass
import concourse.mybir as mybir
from concourse.bass_types import AP
from concourse.tile import TileContext
from concourse.bass_utils import run_bass_kernel_spmd

F32 = mybir.dt.float32
BF16 = mybir.dt.bfloat16
FP8 = mybir.dt.float8e4
AF = mybir.ActivationFunctionType
ALU = mybir.AluOpType

N_CORES = 8
B, T, D = 2048, 128, 512
B_LOC = B // N_CORES          # 256
NH = 2                        # column halves of the local batch
NCH = D // 128                # 4 contraction chunks
# variable DMA chunking: small first chunks land fast (short first-matmul
# latency), big middle chunks stream at higher DMA efficiency, small final
# chunks cut the trailing matmul exposure. Never straddles a 16-t group.
CHUNKS = [(0, 4), (4, 4), (8, 8), (16, 16), (24, 0)]  # placeholder, fixed below
CHUNKS = [(0, 4), (4, 4), (8, 8)] + [(16 * i, 16) for i in range(1, 7)] + \
    [(112, 8), (120, 8)]
NCHUNK = len(CHUNKS)
G = 16                        # timesteps per psum flush group
NGRP = T // G                 # 8
HSTR = T + 2                  # H tile half-stride (even, for 4B alignment)
VMAX = 64                     # max scan window width
SPAN = 4                      # production window = last SPAN flushed groups
TAILS = (96, 112)             # suffix windows after the last flush

_CACHE = {}


def build_nc(w0, w1, w2, bh2):
    nc = bass.Bass(trn_type="TRN2")

    xt = nc.dram_tensor("xt", [128, T, NCH, NH, 128], FP8,
                        kind="ExternalInput")
    w6 = nc.dram_tensor("w6", [128, NCH, 6], FP8, kind="ExternalInput")
    cst = nc.dram_tensor("cst", [128, 52], F32, kind="ExternalInput")
    out = nc.dram_tensor("out", [1, 2 * T], F32, kind="ExternalOutput")

    with TileContext(nc) as tc:
        with (
            tc.tile_pool(name="xpool", bufs=4) as xpool,
            tc.tile_pool(name="consts", bufs=1) as consts,
            tc.tile_pool(name="scan", bufs=1) as scan,
            tc.tile_pool(name="drn", bufs=2) as drn,
            tc.tile_pool(name="swp", bufs=3) as swp,
            tc.tile_pool(name="pmm", bufs=4, space="PSUM") as pmm,
            tc.tile_pool(name="psum2", bufs=1, space="PSUM") as ps2,
        ):
            # ---- constants ----
            # const DMAs ride the ACT HWDGE queue so the Sync queue's
            # FIFO starts with x chunk 0 (transfers per queue are serial)
            w6_sb = consts.tile([128, NCH, 6], FP8)
            nc.scalar.dma_start(out=w6_sb, in_=w6[:])
            cst_sb = consts.tile([128, 52], F32)
            nc.scalar.dma_start(out=cst_sb, in_=cst[:])
            bias48 = cst_sb[:, 0:48]           # cols 3j+g = bias[g]
            h0_sb = cst_sb[:, 48:50]
            ones_sb = consts.tile([128, 1], BF16)
            nc.vector.memset(ones_sb, 1.0)

            # ---- x DMA helper: [128, nt, NCH, NH, 128] fp8, nt KB/partition
            def dma_chunk(c):
                t0, nt = CHUNKS[c]
                x_sb = xpool.tile([128, nt, NCH, NH, 128], FP8,
                                  tag=f"x{nt}", name=f"x_sb{nt}")
                nc.sync.dma_start(out=x_sb, in_=xt[:, t0:t0 + nt])
                return x_sb

            # prefetch exactly 1 chunk so chunk 0 gets the full DMA
            # bandwidth (concurrent transfers round-robin at packet
            # granularity and would delay the first matmul)
            x_tiles = {0: dma_chunk(0)}

            # warm-up consumers of const DMAs (absorb semaphores)
            warm_sb = consts.tile([3, 1], F32)
            nc.scalar.copy(warm_sb, cst_sb[0:3, 50:51])
            warm_dv = consts.tile([3, 1], F32)
            nc.vector.tensor_copy(warm_dv, cst_sb[0:3, 50:51])
            warm_pl = consts.tile([3, 1], F32)
            nc.gpsimd.tensor_copy(warm_pl, cst_sb[0:3, 50:51])

            # ---- persistent buffers ----
            # gi planes [128, 3, NH, T] bf16; H [128, NH*(T+2)] bf16,
            # H col h*HSTR + j holds h_{j-1} (col 0 = h0).
            gi = scan.tile([128, 3, NH, T], BF16)
            H = scan.tile([128, NH * HSTR], BF16)
            nc.gpsimd.memset(H, 0.0)
            h0v = AP(tensor=H.tensor, offset=H.offset, ap=[H.ap[0], [HSTR, NH]])
            nc.vector.tensor_copy(h0v, h0_sb)

            def hview(t0, V):
                # lagged-h view [128, NH, V]: h_{t-1} for t in [t0, t0+V)
                return AP(tensor=H.tensor, offset=H.offset + t0,
                          ap=[H.ap[0], [HSTR, NH], [1, V]])

            def gview(g, t0, V):
                # gate plane view [128, NH, V]
                return AP(tensor=gi.tensor, offset=gi.offset + g * NH * T + t0,
                          ap=[gi.ap[0], [T, NH], [1, V]])

            # ---- production ----
            ps_of = {}     # group -> [ps_h0, ps_h1] (separate banks)

            def produce_chunk(c):
                x_sb = x_tiles.pop(c)
                t0, nt = CHUNKS[c]
                grp = t0 // G
                if t0 % G == 0:
                    ps_of[grp] = [
                        pmm.tile([128, 512], F32, tag="pmm",
                                 name=f"ps{grp}h{h}")
                        for h in range(NH)
                    ]
                pss = ps_of[grp]
                for tt in range(nt):
                    j = t0 + tt - G * grp
                    for k in range(NCH):
                        for h in range(NH):
                            nc.tensor.matmul(
                                pss[h][:, 6 * j:6 * j + 6],
                                x_sb[:, tt, k, h, :],
                                w6_sb[:, k, :],
                                start=(k == 0),
                                stop=(k == NCH - 1),
                            )

            def flush_group(grp):
                pss = ps_of.pop(grp)
                for h in range(NH):
                    ps = pss[h]
                    hi = AP(tensor=ps.tensor, offset=ps.offset,
                            ap=[ps.ap[0], [1, 3], [6, G]])
                    res = AP(tensor=ps.tensor, offset=ps.offset + 3,
                             ap=[ps.ap[0], [1, 3], [6, G]])
                    bias_v = AP(tensor=bias48.tensor, offset=bias48.offset,
                                ap=[bias48.ap[0], [1, 3], [3, G]])
                    sc = drn.tile([128, 48], F32, name="drsc")
                    scv = AP(tensor=sc.tensor, offset=sc.offset,
                             ap=[sc.ap[0], [1, 3], [3, G]])
                    nc.vector.scalar_tensor_tensor(
                        scv, res, 1.0 / 16, bias_v, op0=ALU.mult, op1=ALU.add)
                    giv = AP(tensor=gi.tensor,
                             offset=gi.offset + h * T + G * grp,
                             ap=[gi.ap[0], [NH * T, 3], [1, G]])
                    nc.vector.tensor_tensor(giv, scv, hi, op=ALU.add)

            # ---- one Newton-linearized iteration over window [t0, t1) ----
            def iterate(t0, t1):
                V = t1 - t0
                HL = hview(t0, V)

                def half3(tile):
                    return AP(tensor=tile.tensor, offset=tile.offset,
                              ap=[tile.ap[0], [V, NH], [1, V]])

                srz = swp.tile([128, 4 * VMAX], BF16, tag="srz", name="srz")
                srv = AP(tensor=srz.tensor, offset=srz.offset,
                         ap=[srz.ap[0], [V, NH], [1, V]])
                szv = AP(tensor=srz.tensor, offset=srz.offset + 2 * V,
                         ap=[srz.ap[0], [V, NH], [1, V]])
                nc.vector.scalar_tensor_tensor(
                    srv, HL, w0, gview(0, t0, V), op0=ALU.mult, op1=ALU.add)
                nc.vector.scalar_tensor_tensor(
                    szv, HL, w1, gview(1, t0, V), op0=ALU.mult, op1=ALU.add)
                rz = swp.tile([128, 4 * VMAX], BF16, tag="rz", name="rz")
                nc.scalar.activation(out=rz[:, 0:4 * V], in_=srz[:, 0:4 * V],
                                     func=AF.Sigmoid)
                # flat [128, 2V] views (step-1 bf16 -> DVE 2x mode); the
                # [NH, V] layouts of these tiles are contiguous in cols
                rf = rz[:, 0:2 * V]
                zf = rz[:, 2 * V:4 * V]

                def t2(tag):
                    tl = swp.tile([128, 2 * VMAX], BF16, tag=tag, name=tag)
                    return tl[:, 0:2 * V], half3(tl)

                gh2f, gh2 = t2("gh2")
                nc.gpsimd.tensor_scalar(gh2, HL, w2, bh2,
                                        op0=ALU.mult, op1=ALU.add)
                nrf, nr = t2("nr")
                nc.gpsimd.tensor_tensor(nrf, rf, gh2f, op=ALU.mult)
                npf, npre = t2("npre")
                nc.gpsimd.tensor_tensor(npre, nr, gview(2, t0, V), op=ALU.add)
                ntf, nt = t2("nt")
                nc.scalar.activation(out=ntf, in_=npf, func=AF.Tanh)
                # u = z(1-z)  (z - z^2)
                zzf, zz = t2("zz")
                nc.gpsimd.tensor_tensor(zzf, zf, zf, op=ALU.mult)
                uf, u = t2("u")
                nc.gpsimd.tensor_tensor(uf, zf, zzf, op=ALU.subtract)
                # d = h - n; p = u*d
                df, d = t2("d")
                nc.vector.tensor_tensor(d, HL, nt, op=ALU.subtract)
                pf, p = t2("p")
                nc.vector.tensor_tensor(pf, uf, df, op=ALU.mult)
                phf, pH = t2("pH")
                nc.vector.tensor_tensor(pH, p, HL, op=ALU.mult)
                # a = z + w1*p
                af, av = t2("a")
                nc.vector.scalar_tensor_tensor(af, pf, w1, zf,
                                               op0=ALU.mult, op1=ALU.add)
                # c = n(1-z) - w1*pH
                c1f, c1 = t2("c1")
                nc.gpsimd.tensor_tensor(c1f, zf, ntf, op=ALU.mult)
                c2f, c2 = t2("c2")
                nc.vector.tensor_tensor(c2f, ntf, c1f, op=ALU.subtract)
                cf, cv = t2("c")
                nc.vector.scalar_tensor_tensor(cf, phf, -w1, c2f,
                                               op0=ALU.mult, op1=ALU.add)
                # per-half linear scan: h_t = a_t * h_{t-1} + c_t
                for h in range(NH):
                    nc.vector.tensor_tensor_scan(
                        AP(tensor=H.tensor,
                           offset=H.offset + h * HSTR + t0 + 1,
                           ap=[H.ap[0], [1, V]]),
                        af[:, h * V:(h + 1) * V],
                        cf[:, h * V:(h + 1) * V],
                        AP(tensor=H.tensor, offset=H.offset + h * HSTR + t0,
                           ap=[H.ap[0], [1, 1]]),
                        op0=ALU.mult,
                        op1=ALU.add,
                    )

            # ---- schedule ----
            for c in range(NCHUNK):
                if c + 1 < NCHUNK:
                    x_tiles[c + 1] = dma_chunk(c + 1)
                produce_chunk(c)
                t0, nt = CHUNKS[c]
                if (t0 + nt) % G == 0:
                    grp = (t0 + nt) // G - 1
                    flush_group(grp)
                    if grp < NGRP - 1:
                        t1 = G * (grp + 1)
                        iterate(max(0, t1 - G * SPAN), t1)
                    if grp == NGRP - 2:
                        # extra catch-up window, hidden under production
                        iterate(64, 112)
            for t0 in TAILS:
                iterate(t0, T)

            # ---- batch-sum over partitions ----
            hv = AP(tensor=H.tensor, offset=H.offset + 1,
                    ap=[H.ap[0], [HSTR, NH], [1, T]])
            sum_ps = ps2.tile([1, 2 * T], F32, name="sum_ps")
            nc.tensor.matmul(sum_ps, ones_sb, hv, start=True, stop=True)
            sum_sb = scan.tile([1, 2 * T], F32)
            nc.vector.tensor_copy(sum_sb, sum_ps)
            nc.sync.dma_start(out=out[:], in_=sum_sb)

    _strip_same_engine_waits(nc)
    return nc


_ENG_PFX = {
    mybir.EngineType.Activation: "Activation",
    mybir.EngineType.DVE: "DVE",
    mybir.EngineType.PE: "PE",
    mybir.EngineType.Pool: "Pool",
    mybir.EngineType.SP: "SP",
}


def _strip_same_engine_waits(nc):
    """The compute-engine instruction formats have a single sync-wait slot.

    Tile's semaphore assignment is not transitively minimal and often adds a
    wait on the instruction's own engine semaphore next to a cross-engine
    wait. Engines execute their own stream in order, so same-engine waits
    are vacuous -- drop them when an instruction carries more than one wait.
    """
    multi = []
    for inst in nc.inst_map.values():
        si = inst.sync_info
        if not si or not si.on_wait or len(si.on_wait) <= 1:
            continue
        pfx = _ENG_PFX.get(inst.engine)
        if pfx is not None:
            kept = [
                w
                for w in si.on_wait
                if not (w.ant_name or "").startswith(pfx + "_")
            ]
            if len(kept) != len(si.on_wait):
                si.on_wait = kept
        if len(si.on_wait) > 1:
            multi.append((inst.name, type(inst).__name__, str(inst.engine),
                          [w.ant_name for w in si.on_wait]))

    # Any instruction still carrying >1 wait cannot encode (single HW wait
    # slot): hoist all but one wait onto single-wait InstDrains inserted
    # just before it on the same engine.
    for block in nc.m.functions[0].blocks:
        insts = block.instructions
        for idx in range(len(insts) - 1, -1, -1):
            inst = insts[idx]
            si = inst.sync_info
            if not si or not si.on_wait or len(si.on_wait) <= 1:
                continue
            waits = list(si.on_wait)
            si.on_wait = waits[-1:]
            pre = []
            for k, w in enumerate(waits[:-1]):
                d = mybir.InstDrain(
                    name=f"{inst.name}-w{k}", ins=[], outs=[]
                )
                d.engine = inst.engine
                d.sync_info = mybir.SyncInfo(on_wait=[w], on_update=[])
                pre.append(d)
            insts[idx:idx] = pre
            multi = [m for m in multi if m[0] != inst.name]

    if multi:
        import sys
        print(f"[kernel] WARNING: {len(multi)} instructions still have >1 "
              f"sync wait: {multi[:8]}", file=sys.stderr)


def kernel(x, h0, w_ih, w_hh, b_ih, b_hh):
    import ml_dtypes
    fp8 = ml_dtypes.float8_e4m3fn

    x = np.asarray(x, dtype=np.float32)
    h0 = np.asarray(h0, dtype=np.float32)
    w_ih = np.asarray(w_ih, dtype=np.float32)
    w_hh = np.asarray(w_hh, dtype=np.float32)
    b_ih = np.asarray(b_ih, dtype=np.float32)
    b_hh = np.asarray(b_hh, dtype=np.float32)

    w0, w1, w2 = (float(v) for v in w_hh[:, 0])
    bh0, bh1, bh2 = (float(v) for v in b_hh)
    key = (w0, w1, w2, bh2)
    if _CACHE.get("key") != key:
        _CACHE["nc"] = build_nc(w0, w1, w2, bh2)
        _CACHE["key"] = key
    nc = _CACHE["nc"]

    # w6[p, k, 0:3] = fp8(w_ih[g, 128k+p]); w6[p, k, 3:6] = fp8(16*residual)
    w_hi = w_ih.astype(fp8).astype(np.float32)
    w_res = ((w_ih - w_hi) * 16.0).astype(fp8).astype(np.float32)
    w6 = np.zeros((128, NCH, 6), dtype=np.float32)
    w6[:, :, 0:3] = w_hi.T.reshape(NCH, 128, 3).transpose(1, 0, 2)
    w6[:, :, 3:6] = w_res.T.reshape(NCH, 128, 3).transpose(1, 0, 2)
    w6 = w6.astype(fp8)

    bias3 = np.array([b_ih[0] + bh0, b_ih[1] + bh1, b_ih[2]], dtype=np.float32)

    in_maps = []
    for c in range(N_CORES):
        xs = x[c * B_LOC:(c + 1) * B_LOC]                 # [B_loc, T, D]
        # [dpart, t, k, h, b]; xs b_loc index = 128h + b
        xtb = np.ascontiguousarray(
            xs.reshape(NH, 128, T, NCH, 128).transpose(4, 2, 3, 0, 1)
        ).astype(fp8)
        h0c = h0[0, c * B_LOC:(c + 1) * B_LOC, 0]         # [B_loc]
        h0t = h0c.reshape(NH, 128).T                      # [128, NH]
        cstc = np.zeros((128, 52), dtype=np.float32)
        cstc[:, 0:48] = np.tile(bias3, 16)
        cstc[:, 48:50] = h0t
        in_maps.append({"xt": xtb, "w6": w6, "cst": cstc})

    res = run_bass_kernel_spmd(nc, in_maps, core_ids=list(range(N_CORES)))
    total = np.zeros((2 * T,), dtype=np.float64)
    for r in res.results:
        total += r["out"].reshape(-1).astype(np.float64)
    out = total.reshape(NH, T).sum(axis=0) / B
    return out.astype(np.float32)
